# revision 10
# baseline (speedup 1.0000x reference)
"""Trainium2 Bass kernel for BatchWiseTripletDistanceLoss.

Math: loss = mean_t relu(cos_d(s[a_t], s[p_t]) - cos_d(s[a_t], n_t]) + margin)
with cos_d(x, y) = 1 - <x,y>/max(|x||y|, eps).

Since cos distances depend only on (row, row) pairs of the 512x256 sample
matrix, the kernel computes the 512x512 cosine-SIMILARITY matrix
sim = R S S^T R (R = diag(1/|s_i|)) on-device via TensorE, and evaluates
    relu(sim[a,p] - sim[a,n] + margin)          ("1-" cancels in the diff)
on a dense [row, col] grid: each triplet is scattered to grid cell
(a_t, n_t) carrying its p_t (gpsimd local_scatter does a true
per-partition scatter).  The per-row positive candidates (same label,
index > row) form a small palette (<= max class size) derived on-device
from `targets`; a short loop over palette slots turns the per-cell p_t
into its sim value via equality masks, so no per-triplet gather is ever
needed.

Sharding: 8 cores split the grid into (row mod 4) x (column half)
quadrants of [128, 256]; the host only buckets/pads/permutes the raw
index arrays per core and sums the 8 partial scalars at the end.
"""
import sys

sys.path.insert(0, "/opt/trn_rl_repo")

from contextlib import ExitStack

import numpy as np

import concourse.bacc as bacc
import concourse.bass as bass
import concourse.masks as masks
import concourse.tile as tile
from concourse import mybir
from concourse.bass_utils import run_bass_kernel_spmd

DT = mybir.dt
OP = mybir.AluOpType
ACTF = mybir.ActivationFunctionType

N = 512
D = 256
MARGIN = 0.15
NCORES = 8
LCOL = 256  # columns per core (half)
NROW = 128  # rows per core (stride-4 residue class)


def _build_program(s_pad: int, stage: str = "full"):
    """Build + compile the SPMD program (same for all 8 cores)."""
    nc = bacc.Bacc(
        "TRN2", target_bir_lowering=False, debug=False, num_devices=NCORES
    )
    f32, i32, i16, f16 = DT.float32, DT.int32, DT.int16, DT.float16

    d_samp = nc.dram_tensor("samples_perm", [N, D], f32, kind="ExternalInput").ap()
    d_rows = nc.dram_tensor("samples_rows", [NROW, D], f32, kind="ExternalInput").ap()
    d_tperm = nc.dram_tensor("targets_perm", [1, N], i32, kind="ExternalInput").ap()
    d_iraw = nc.dram_tensor("iotaraw", [1, N], i32, kind="ExternalInput").ap()
    d_labc = nc.dram_tensor("labcol", [NROW, 1], i32, kind="ExternalInput").ap()
    d_rowid = nc.dram_tensor("rowidf", [NROW, 1], f32, kind="ExternalInput").ap()
    d_nbuk = nc.dram_tensor("nbuk", [NROW, LCOL], i32, kind="ExternalInput").ap()
    d_pbuk = nc.dram_tensor("pbuk", [NROW, LCOL], i32, kind="ExternalInput").ap()
    d_out = nc.dram_tensor("out", [1, 1], f32, kind="ExternalOutput").ap()

    with tile.TileContext(nc) as tc, ExitStack() as ctx:
        cpool = ctx.enter_context(tc.tile_pool(name="const", bufs=1))
        wpool = ctx.enter_context(tc.tile_pool(name="work", bufs=2))
        mpool = ctx.enter_context(tc.tile_pool(name="mainloop", bufs=4))
        ppool = ctx.enter_context(tc.tile_pool(name="psum", bufs=2, space="PSUM"))
        pfin = ctx.enter_context(tc.tile_pool(name="psumfin", bufs=1, space="PSUM"))
        pbig = ctx.enter_context(tc.tile_pool(name="psumbig", bufs=1, space="PSUM"))

        # ---- load inputs -------------------------------------------------
        sp = []
        for m in range(4):
            t = cpool.tile([128, D], f32, tag=f"sp{m}", name=f"sp{m}")
            nc.sync.dma_start(t[:], d_samp[128 * m : 128 * (m + 1), :])
            sp.append(t)
        sr = cpool.tile([NROW, D], f32)
        nc.sync.dma_start(sr[:], d_rows)
        tperm32 = cpool.tile([1, N], i32)
        nc.sync.dma_start(tperm32[:], d_tperm)
        iraw32 = cpool.tile([1, N], i32)
        nc.sync.dma_start(iraw32[:], d_iraw)
        labc32 = cpool.tile([NROW, 1], i32)
        nc.sync.dma_start(labc32[:], d_labc)
        rowidf = cpool.tile([NROW, 1], f32)
        nc.sync.dma_start(rowidf[:], d_rowid)
        nbuk = cpool.tile([NROW, LCOL], i32)
        nc.sync.dma_start(nbuk[:], d_nbuk)
        pbuk = cpool.tile([NROW, LCOL], i32)
        nc.sync.dma_start(pbuk[:], d_pbuk)

        ident = cpool.tile([128, 128], f32)
        masks.make_identity(nc, ident[:])
        ones_col = cpool.tile([128, 1], f32)
        nc.vector.memset(ones_col[:], 1.0)
        ones_row1 = cpool.tile([1, 128], f32)
        nc.vector.memset(ones_row1[:], 1.0)
        zero_row = cpool.tile([128, N], f32)
        nc.gpsimd.memset(zero_row[:], 0.0)

        # ---- transpose samples_perm -> ST_k [128(d), 512(col)] ----------
        st = [cpool.tile([128, N], f32, tag=f"st{k}", name=f"st{k}") for k in range(2)]
        for m in range(4):
            for k in range(2):
                pt = ppool.tile([128, 128], f32, tag="tp")
                nc.tensor.transpose(pt[:], sp[m][:, 128 * k : 128 * (k + 1)], ident[:])
                nc.scalar.copy(st[k][:, 128 * m : 128 * (m + 1)], pt[:])
        # transpose samples_rows -> srt_k [128(d), 128(row)]
        srt = [cpool.tile([128, 128], f32, tag=f"srt{k}", name=f"srt{k}") for k in range(2)]
        for k in range(2):
            pt = ppool.tile([128, 128], f32, tag="tp")
            nc.tensor.transpose(pt[:], sr[:, 128 * k : 128 * (k + 1)], ident[:])
            nc.scalar.copy(srt[k][:], pt[:])

        # ---- column norms -> rrow [1, 512] = 1/|s_col| -------------------
        sq = wpool.tile([128, N], f32, tag="sq")
        n2p = pbig.tile([1, N], f32, tag="n2row")
        for k in range(2):
            nc.vector.tensor_tensor(sq[:], st[k][:], st[k][:], OP.mult)
            nc.tensor.matmul(
                n2p[:], ones_col[:], sq[:], start=(k == 0), stop=(k == 1)
            )
        nrow = wpool.tile([1, N], f32, tag="nrow")
        nc.scalar.sqrt(nrow[:], n2p[:])
        rrow = cpool.tile([1, N], f32)
        nc.vector.reciprocal(rrow[:], nrow[:])

        # ---- row norms -> rr [128, 1] = 1/|s_row| ------------------------
        sqr = wpool.tile([NROW, D], f32, tag="sqr")
        nc.vector.tensor_tensor(sqr[:], sr[:], sr[:], OP.mult)
        n2r = wpool.tile([NROW, 1], f32, tag="n2r")
        nc.vector.tensor_reduce(n2r[:], sqr[:], mybir.AxisListType.X, OP.add)
        nrr = wpool.tile([NROW, 1], f32, tag="nrr")
        nc.scalar.sqrt(nrr[:], n2r[:])
        rr = cpool.tile([NROW, 1], f32)
        nc.vector.reciprocal(rr[:], nrr[:])

        # ---- scale ST columns by rrow: need rrow broadcast to [128, N] --
        rbp = pbig.tile([128, N], f32, tag="rb")
        nc.tensor.matmul(rbp[:], ones_row1[:], rrow[:], start=True, stop=True)
        for k in range(2):
            nc.vector.tensor_tensor(st[k][:], st[k][:], rbp[:], OP.mult)

        # ---- sim rows: psum [128, 512] = srt^T @ st' ; scale rows by rr --
        simp = pbig.tile([128, N], f32, tag="simp")
        for k in range(2):
            nc.tensor.matmul(simp[:], srt[k][:], st[k][:], start=(k == 0), stop=(k == 1))
        simrow = cpool.tile([128, N], f32)
        nc.scalar.activation(simrow[:], simp[:], ACTF.Copy, scale=rr[:])
        sim16 = cpool.tile([128, N], f16)
        nc.scalar.copy(sim16[:], simrow[:])

        if stage == "sim":
            outs = wpool.tile([1, 1], f32, tag="outs")
            nc.scalar.copy(outs[:], simrow[0:1, 0:1])
            nc.sync.dma_start(d_out, outs[:])
            nc.compile()
            return nc
        # ---- label / column-id broadcasts -------------------------------
        tpermf1 = wpool.tile([1, N], f32, tag="tpf")
        nc.scalar.copy(tpermf1[:], tperm32[:])
        irawf1 = wpool.tile([1, N], f32, tag="irf")
        nc.scalar.copy(irawf1[:], iraw32[:])
        labp = pbig.tile([128, N], f32, tag="labrow")
        nc.tensor.matmul(labp[:], ones_row1[:], tpermf1[:], start=True, stop=True)
        irap = pbig.tile([128, N], f32, tag="irarow")
        nc.tensor.matmul(irap[:], ones_row1[:], irawf1[:], start=True, stop=True)
        labcf = wpool.tile([NROW, 1], f32, tag="labcf")
        nc.vector.tensor_copy(labcf[:], labc32[:])

        # ---- positive-candidate mask + ranks ----------------------------
        m1 = wpool.tile([128, N], f32, tag="m1")
        nc.vector.tensor_scalar(m1[:], labp[:], labcf[:, 0:1], None, OP.is_equal)
        m2 = wpool.tile([128, N], f32, tag="m2")
        nc.vector.tensor_scalar(m2[:], irap[:], rowidf[:, 0:1], None, OP.is_gt)
        pm = wpool.tile([128, N], f32, tag="pm")
        nc.vector.tensor_tensor(pm[:], m1[:], m2[:], OP.mult)
        incl = wpool.tile([128, N], f32, tag="incl")
        if stage == "noscan":
            nc.vector.tensor_copy(incl[:], pm[:])
        else:
            nc.vector.tensor_tensor_scan(incl[:], pm[:], zero_row[:], 0.0, OP.add, OP.add)
        sidxf = wpool.tile([128, N], f32, tag="sidxf")
        nc.vector.tensor_tensor(sidxf[:], pm[:], incl[:], OP.mult)
        sidx16 = wpool.tile([128, N], i16, tag="sidx16")
        nc.vector.tensor_scalar(sidx16[:], sidxf[:], -1.0, None, OP.add)
        iota16 = wpool.tile([128, N], i16, tag="iota16")
        nc.vector.tensor_copy(iota16[:], irap[:])

        if stage == "scan":
            outs = wpool.tile([1, 1], f32, tag="outs")
            nc.scalar.copy(outs[:], sidxf[0:1, 0:1])
            nc.sync.dma_start(d_out, outs[:])
            nc.compile()
            return nc
        # ---- palettes via per-partition local scatters ------------------
        palidx = wpool.tile([128, s_pad], i16, tag="palidx")
        palv16 = wpool.tile([128, s_pad], f16, tag="palv16")
        if stage == "noscat":
            nc.vector.memset(palidx[:], 1)
            nc.vector.memset(palv16[:], 0.5)
        else:
            nc.gpsimd.local_scatter(
                palidx[:], iota16[:], sidx16[:],
                channels=128, num_elems=s_pad, num_idxs=N,
            )
            nc.gpsimd.local_scatter(
                palv16[:], sim16[:], sidx16[:],
                channels=128, num_elems=s_pad, num_idxs=N,
            )
        palidx1f = wpool.tile([128, s_pad], f32, tag="palidx1f")
        nc.vector.tensor_scalar(palidx1f[:], palidx[:], 1, None, OP.add)
        # mb[:, s] = margin - palv_s
        mb = wpool.tile([128, s_pad], f32, tag="mb")
        nc.vector.tensor_scalar(mb[:], palv16[:], -1.0, MARGIN, OP.mult, OP.add)

        if stage == "pal":
            outs = wpool.tile([1, 1], f32, tag="outs")
            nc.scalar.copy(outs[:], mb[0:1, 0:1])
            nc.sync.dma_start(d_out, outs[:])
            nc.compile()
            return nc
        # ---- scatter p+1 to grid cells (row, ncol) ----------------------
        nf = wpool.tile([NROW, LCOL], f32, tag="nf")
        nc.vector.tensor_copy(nf[:], nbuk[:])
        ge = wpool.tile([NROW, LCOL], f32, tag="ge")
        nc.vector.tensor_scalar(ge[:], nf[:], 256.0, None, OP.is_ge)
        nidxf = wpool.tile([NROW, LCOL], f32, tag="nidxf")
        nc.vector.scalar_tensor_tensor(
            nidxf[:], ge[:], -256.0, nf[:], OP.mult, OP.add
        )
        nidx16 = wpool.tile([NROW, LCOL], i16, tag="nidx16")
        nc.vector.tensor_copy(nidx16[:], nidxf[:])
        p16 = wpool.tile([NROW, LCOL], i16, tag="p16")
        nc.vector.tensor_scalar(p16[:], pbuk[:], 1, None, OP.add)
        pidxg = wpool.tile([NROW, LCOL], i16, tag="pidxg")
        if stage == "noscat":
            nc.vector.memset(pidxg[:], 2)
        else:
            nc.gpsimd.local_scatter(
                pidxg[:], p16[:], nidx16[:],
                channels=128, num_elems=LCOL, num_idxs=LCOL,
            )
        pidxgf = wpool.tile([NROW, LCOL], f32, tag="pidxgf")
        nc.vector.tensor_copy(pidxgf[:], pidxg[:])

        if stage == "pidxg":
            pxf = wpool.tile([1, 1], f32, tag="pxf")
            nc.vector.tensor_copy(pxf[:], pidxg[0:1, 0:1])
            nc.sync.dma_start(d_out, pxf[:])
            nc.compile()
            return nc
        # ---- main palette loop ------------------------------------------
        grid = simrow[:, 0:LCOL]
        acc = wpool.tile([128, s_pad], f32, tag="acc")
        for s in range(s_pad):
            ms = mpool.tile([128, LCOL], f32, tag="ms", name=f"ms{s}")
            nc.vector.tensor_scalar(
                ms[:], pidxgf[:], palidx1f[:, s : s + 1], None, OP.is_equal
            )
            mskd = mpool.tile([128, LCOL], f32, tag="mskd", name=f"mk{s}")
            nc.vector.scalar_tensor_tensor(
                mskd[:], grid, mb[:, s : s + 1], ms[:], OP.add, OP.mult
            )
            rl = mpool.tile([128, LCOL], f32, tag="rl", name=f"rl{s}")
            nc.scalar.activation(
                rl[:], mskd[:], ACTF.Relu, accum_out=acc[:, s : s + 1]
            )
        accr = wpool.tile([128, 1], f32, tag="accr")
        nc.vector.tensor_reduce(accr[:], acc[:], mybir.AxisListType.X, OP.add)

        # ---- total: sum across partitions via ones matmul ---------------
        fin = pfin.tile([1, 1], f32, tag="fin")
        nc.tensor.matmul(fin[:], accr[:], ones_col[:], start=True, stop=True)
        outs = wpool.tile([1, 1], f32, tag="outs")
        nc.scalar.copy(outs[:], fin[:])
        nc.sync.dma_start(d_out, outs[:])

    nc.compile()
    return nc


_PROGRAM_CACHE = {}


def _get_program(s_pad):
    if s_pad not in _PROGRAM_CACHE:
        _PROGRAM_CACHE[s_pad] = _build_program(s_pad)
    return _PROGRAM_CACHE[s_pad]


def _shard_inputs(samples, targets, a, p, n):
    """Bucket triplets into per-core [128, 256] scatter arrays."""
    in_maps = []
    for core in range(NCORES):
        R, H = core >> 1, core & 1
        rows = np.arange(NROW, dtype=np.int64) * 4 + R
        perm = np.concatenate(
            [np.arange(256 * H, 256 * H + 256), np.arange(256 * (1 - H), 256 * (2 - H))]
        )
        sel = ((a & 3) == R) & ((n >> 8) == H)
        asel, psel, nsel = a[sel], p[sel], n[sel]
        q = asel >> 2
        # stable-sort by partition, sequential slot within partition
        order = np.argsort(q, kind="stable")
        qs = q[order]
        counts = np.bincount(qs, minlength=NROW)
        if counts.max() > LCOL:
            raise ValueError("bucket overflow")
        starts = np.zeros(NROW, dtype=np.int64)
        starts[1:] = np.cumsum(counts)[:-1]
        slot = np.arange(len(qs)) - starts[qs]
        nbuk = np.full((NROW, LCOL), -1, dtype=np.int32)
        pbuk = np.zeros((NROW, LCOL), dtype=np.int32)
        nbuk[qs, slot] = nsel[order]
        pbuk[qs, slot] = psel[order]
        in_maps.append(
            {
                "samples_perm": np.ascontiguousarray(samples[perm]),
                "samples_rows": np.ascontiguousarray(samples[rows]),
                "targets_perm": targets[perm].reshape(1, N).astype(np.int32),
                "iotaraw": perm.reshape(1, N).astype(np.int32),
                "labcol": targets[rows].reshape(NROW, 1).astype(np.int32),
                "rowidf": rows.reshape(NROW, 1).astype(np.float32),
                "nbuk": nbuk,
                "pbuk": pbuk,
            }
        )
    return in_maps


def kernel(samples, targets, anchor_idx, pos_idx, neg_idx, _want_trace=False):
    samples = np.asarray(samples, dtype=np.float32)
    targets = np.asarray(targets).astype(np.int32)
    a = np.asarray(anchor_idx).astype(np.int64)
    p = np.asarray(pos_idx).astype(np.int64)
    n = np.asarray(neg_idx).astype(np.int64)
    T = a.shape[0]
    assert samples.shape == (N, D) and targets.shape == (N,)

    # structural guards (hold for the reference triplet miner)
    ok = (
        np.all((a >= 0) & (a < N) & (p >= 0) & (p < N) & (n >= 0) & (n < N))
        and np.all(targets[p] == targets[a])
        and np.all(p > a)
        and len(np.unique(a * N + n)) == T
    )
    if not ok:
        raise NotImplementedError("inputs violate mined-triplet structure")

    # palette capacity: max count of same-label successors over rows
    order = np.argsort(targets, kind="stable")
    npos = np.array(
        [np.sum((targets == targets[i]) & (np.arange(N) > i)) for i in range(N)]
    )
    s_pad = max(2, int(npos.max()) + (int(npos.max()) & 1))

    nc = _get_program(s_pad)
    in_maps = _shard_inputs(samples, targets, a, p, n)
    res = run_bass_kernel_spmd(
        nc, in_maps, list(range(NCORES)), trace=_want_trace
    )
    total = sum(float(res.results[c]["out"][0, 0]) for c in range(NCORES))
    loss = np.float32(total / T)
    if _want_trace:
        return loss, res
    return loss


# revision 11
# speedup vs baseline: 1.3792x; 1.3792x over previous
"""Trainium2 Bass kernel for BatchWiseTripletDistanceLoss.

Math: loss = mean_t relu(cos_d(s[a_t], s[p_t]) - cos_d(s[a_t], s[n_t]) + margin)
with cos_d(x, y) = 1 - <x,y>/max(|x||y|, eps).

Since cosine distances depend only on (row, row) pairs of the 512x256 sample
matrix, the kernel computes the 512x512 cosine-SIMILARITY matrix
sim = R S S^T R (R = diag(1/|s_i|)) on-device via TensorE, and evaluates
    relu(sim[a,p] - sim[a,n] + margin)          ("1-" cancels in the diff)
on a dense [row, col] grid: each triplet is scattered to grid cell
(a_t, n_t) carrying p_t+1 (gpsimd local_scatter = true per-partition
scatter).  The distinct positives of each row form a small palette
(max ~13 entries here); palette VALUES are extracted from the sim row by a
second local_scatter, and a short loop over palette slots evaluates
masked relu terms, so no per-triplet gather is ever needed.

Sharding: 8 cores split the grid into (row mod 4) x (column half)
quadrants of [128, 256].  The host only transposes/permutes/buckets/pads
the given arrays (layout + palette metadata, no float math) and sums the
8 partial scalars at the end.
"""
import sys

sys.path.insert(0, "/opt/trn_rl_repo")

from contextlib import ExitStack

import numpy as np

import concourse.bacc as bacc
import concourse.bass as bass
import concourse.tile as tile
from concourse import mybir
from concourse.bass_utils import run_bass_kernel_spmd

DT = mybir.dt
OP = mybir.AluOpType
ACTF = mybir.ActivationFunctionType

N = 512
D = 256
MARGIN = 0.15
NCORES = 8
LCOL = 256  # columns per core (half)
NROW = 128  # rows per core (stride-4 residue class)


def _build_program(s_pad: int):
    """Build + compile the SPMD program (identical for all 8 cores)."""
    nc = bacc.Bacc(
        "TRN2", target_bir_lowering=False, debug=False, num_devices=NCORES
    )
    f32, i32, i16, f16 = DT.float32, DT.int32, DT.int16, DT.float16

    d_stp = nc.dram_tensor("stp", [D, N], f32, kind="ExternalInput").ap()
    d_srt = nc.dram_tensor("srt", [D, NROW], f32, kind="ExternalInput").ap()
    d_sidx = nc.dram_tensor("sidx16", [NROW, N], DT.int16, kind="ExternalInput").ap()
    d_pal = nc.dram_tensor("palidx1f", [NROW, s_pad], f32, kind="ExternalInput").ap()
    d_nbuk = nc.dram_tensor("nbuk", [NROW, LCOL], i32, kind="ExternalInput").ap()
    d_pbuk = nc.dram_tensor("pbuk", [NROW, LCOL], i32, kind="ExternalInput").ap()
    d_out = nc.dram_tensor("out", [1, 1], f32, kind="ExternalOutput").ap()

    with tile.TileContext(nc) as tc, ExitStack() as ctx:
        cpool = ctx.enter_context(tc.tile_pool(name="const", bufs=1))
        wpool = ctx.enter_context(tc.tile_pool(name="work", bufs=2))
        mpool = ctx.enter_context(tc.tile_pool(name="mainloop", bufs=4))
        ppool = ctx.enter_context(tc.tile_pool(name="psum", bufs=2, space="PSUM"))
        pfin = ctx.enter_context(tc.tile_pool(name="psumfin", bufs=1, space="PSUM"))
        pbig = ctx.enter_context(tc.tile_pool(name="psumbig", bufs=1, space="PSUM"))

        # ---- load inputs -------------------------------------------------
        st = []
        for k in range(2):
            t = cpool.tile([128, N], f32, tag=f"st{k}", name=f"st{k}")
            nc.sync.dma_start(t[:], d_stp[128 * k : 128 * (k + 1), :])
            st.append(t)
        sr = []
        for k in range(2):
            t = cpool.tile([128, NROW], f32, tag=f"sr{k}", name=f"sr{k}")
            nc.sync.dma_start(t[:], d_srt[128 * k : 128 * (k + 1), :])
            sr.append(t)
        nbuk = cpool.tile([NROW, LCOL], i32)
        nc.sync.dma_start(nbuk[:], d_nbuk)
        pbuk = cpool.tile([NROW, LCOL], i32)
        nc.sync.dma_start(pbuk[:], d_pbuk)
        sidx16 = cpool.tile([NROW, N], DT.int16)
        nc.sync.dma_start(sidx16[:], d_sidx)
        palidx1f = cpool.tile([NROW, s_pad], f32)
        nc.sync.dma_start(palidx1f[:], d_pal)

        ones_col = cpool.tile([128, 1], f32)
        nc.vector.memset(ones_col[:], 1.0)
        ones_row1 = cpool.tile([1, 128], f32)
        nc.vector.memset(ones_row1[:], 1.0)

        # ---- raw similarity matmul (starts as soon as DMAs land) --------
        simp = pbig.tile([128, N], f32, tag="simp")
        for k in range(2):
            nc.tensor.matmul(simp[:], sr[k][:], st[k][:], start=(k == 0), stop=(k == 1))

        # ---- column norms -> rrow [1, 512] = 1/|s_col| -------------------
        sq = wpool.tile([128, N], f32, tag="sq")
        n2p = pbig.tile([1, N], f32, tag="n2row")
        for k in range(2):
            nc.vector.tensor_tensor(sq[:], st[k][:], st[k][:], OP.mult)
            nc.tensor.matmul(n2p[:], ones_col[:], sq[:], start=(k == 0), stop=(k == 1))
        # ---- row norms -> [128, 1] --------------------------------------
        sqr = wpool.tile([128, NROW], f32, tag="sqr")
        n2rp = ppool.tile([128, 1], f32, tag="n2rp")
        for k in range(2):
            nc.vector.tensor_tensor(sqr[:], sr[k][:], sr[k][:], OP.mult)
            nc.tensor.matmul(n2rp[:], sqr[:], ones_col[:], start=(k == 0), stop=(k == 1))

        nrow = wpool.tile([1, N], f32, tag="nrow")
        nc.scalar.sqrt(nrow[:], n2p[:])
        nrr = wpool.tile([128, 1], f32, tag="nrr")
        nc.scalar.sqrt(nrr[:], n2rp[:])
        rrow = wpool.tile([1, N], f32, tag="rrow")
        rscr = wpool.tile([1, N], f32, tag="rscr")
        nc.vector.reciprocal_approx_accurate(rrow[:], nrow[:], rscr[:])
        rr = cpool.tile([128, 1], f32)
        rscr2 = wpool.tile([128, 1], f32, tag="rscr2")
        nc.vector.reciprocal_approx_accurate(rr[:], nrr[:], rscr2[:])

        # ---- combine: simrow = (raw . rr) x RB --------------------------
        rbp = pbig.tile([128, N], f32, tag="rb")
        nc.tensor.matmul(rbp[:], ones_row1[:], rrow[:], start=True, stop=True)
        t0 = wpool.tile([128, N], f32, tag="t0")
        nc.scalar.activation(t0[:], simp[:], ACTF.Copy, scale=rr[:])
        simrow = cpool.tile([128, N], f32)
        nc.vector.tensor_tensor(simrow[:], t0[:], rbp[:], OP.mult)
        sim16 = cpool.tile([128, N], DT.float16)
        nc.scalar.copy(sim16[:], simrow[:])
        grid16 = cpool.tile([128, LCOL], DT.float16)
        nc.scalar.copy(grid16[:], simrow[:, 0:LCOL])

        # ---- palette values + margin bias -------------------------------
        palv16 = wpool.tile([128, s_pad], DT.float16, tag="palv16")
        nc.gpsimd.local_scatter(
            palv16[:], sim16[:], sidx16[:],
            channels=128, num_elems=s_pad, num_idxs=N,
        )
        mb = wpool.tile([128, s_pad], f32, tag="mb")
        nc.vector.tensor_scalar(mb[:], palv16[:], -1.0, MARGIN, OP.mult, OP.add)

        # ---- scatter p+1 to grid cells (row, ncol) ----------------------
        nf = wpool.tile([NROW, LCOL], f32, tag="nf")
        nc.vector.tensor_copy(nf[:], nbuk[:])
        ge = wpool.tile([NROW, LCOL], f32, tag="ge")
        nc.vector.tensor_scalar(ge[:], nf[:], 256.0, None, OP.is_ge)
        nidxf = wpool.tile([NROW, LCOL], f32, tag="nidxf")
        nc.vector.scalar_tensor_tensor(
            nidxf[:], ge[:], -256.0, nf[:], OP.mult, OP.add
        )
        nidx16 = wpool.tile([NROW, LCOL], DT.int16, tag="nidx16")
        nc.vector.tensor_copy(nidx16[:], nidxf[:])
        p16 = wpool.tile([NROW, LCOL], DT.float16, tag="p16")
        nc.vector.tensor_scalar(p16[:], pbuk[:], 1, None, OP.add)
        pidxg = wpool.tile([NROW, LCOL], DT.float16, tag="pidxg")
        nc.gpsimd.local_scatter(
            pidxg[:], p16[:], nidx16[:],
            channels=128, num_elems=LCOL, num_idxs=LCOL,
        )

        # ---- main palette loop ------------------------------------------
        acc = wpool.tile([128, s_pad], f32, tag="acc")
        for s in range(s_pad):
            ms = mpool.tile([128, LCOL], DT.float16, tag="ms", name=f"ms{s}")
            nc.vector.tensor_scalar(
                ms[:], pidxg[:], palidx1f[:, s : s + 1], None, OP.is_equal
            )
            mskd = mpool.tile([128, LCOL], DT.float16, tag="mskd", name=f"mk{s}")
            nc.vector.scalar_tensor_tensor(
                mskd[:], grid16[:], mb[:, s : s + 1], ms[:], OP.add, OP.mult
            )
            rl = mpool.tile([128, LCOL], DT.float16, tag="rl", name=f"rl{s}")
            nc.scalar.activation(
                rl[:], mskd[:], ACTF.Relu, accum_out=acc[:, s : s + 1]
            )
        accr = wpool.tile([128, 1], f32, tag="accr")
        nc.vector.tensor_reduce(accr[:], acc[:], mybir.AxisListType.X, OP.add)

        # ---- total: sum across partitions via ones matmul ---------------
        fin = pfin.tile([1, 1], f32, tag="fin")
        nc.tensor.matmul(fin[:], accr[:], ones_col[:], start=True, stop=True)
        outs = wpool.tile([1, 1], f32, tag="outs")
        nc.scalar.copy(outs[:], fin[:])
        nc.sync.dma_start(d_out, outs[:])

    nc.compile()
    return nc


_PROGRAM_CACHE = {}


def _get_program(s_pad):
    if s_pad not in _PROGRAM_CACHE:
        _PROGRAM_CACHE[s_pad] = _build_program(s_pad)
    return _PROGRAM_CACHE[s_pad]


def _shard_inputs(samples, targets, a, p, n, s_pad):
    """Per-core layout: transpose/permute samples, bucket triplets, build
    palette metadata (distinct positives per row)."""
    in_maps = []
    for core in range(NCORES):
        R, H = core >> 1, core & 1
        rows = np.arange(NROW, dtype=np.int64) * 4 + R
        perm = np.concatenate(
            [np.arange(256 * H, 256 * H + 256), np.arange(256 * (1 - H), 256 * (2 - H))]
        )
        sel = ((a & 3) == R) & ((n >> 8) == H)
        asel, psel, nsel = a[sel], p[sel], n[sel]
        q = asel >> 2
        order = np.argsort(q, kind="stable")
        qs = q[order]
        counts = np.bincount(qs, minlength=NROW)
        if counts.max() > LCOL:
            raise ValueError("bucket overflow")
        starts = np.zeros(NROW, dtype=np.int64)
        starts[1:] = np.cumsum(counts)[:-1]
        slot = np.arange(len(qs)) - starts[qs]
        nbuk = np.full((NROW, LCOL), -1, dtype=np.int32)
        pbuk = np.zeros((NROW, LCOL), dtype=np.int32)
        nbuk[qs, slot] = nsel[order]
        pbuk[qs, slot] = psel[order]

        # palettes: distinct positives per row; local col of raw id v:
        # (v & 255) + 256 * (v >> 8 != H)
        sidx = np.full((NROW, N), -1, dtype=np.int16)
        palidx1 = np.full((NROW, s_pad), 1.0, dtype=np.float32)  # 1 matches nothing
        ar = a[(a & 3) == R]
        pr = p[(a & 3) == R]
        rr_ = ar >> 2
        for qq in range(NROW):
            vals = np.unique(pr[rr_ == qq])
            if len(vals) > s_pad:
                raise ValueError("palette overflow")
            if len(vals) == 0:
                continue
            lcols = (vals & 255) + 256 * ((vals >> 8) != H)
            sidx[qq, lcols] = np.arange(len(vals), dtype=np.int16)
            palidx1[qq, : len(vals)] = vals + 1.0
        in_maps.append(
            {
                "stp": np.ascontiguousarray(samples[perm].T),
                "srt": np.ascontiguousarray(samples[rows].T),
                "sidx16": sidx,
                "palidx1f": palidx1,
                "nbuk": nbuk,
                "pbuk": pbuk,
            }
        )
    return in_maps


def kernel(samples, targets, anchor_idx, pos_idx, neg_idx, _want_trace=False):
    samples = np.asarray(samples, dtype=np.float32)
    targets = np.asarray(targets).astype(np.int32)
    a = np.asarray(anchor_idx).astype(np.int64)
    p = np.asarray(pos_idx).astype(np.int64)
    n = np.asarray(neg_idx).astype(np.int64)
    T = a.shape[0]
    assert samples.shape == (N, D)

    ok = (
        np.all((a >= 0) & (a < N) & (p >= 0) & (p < N) & (n >= 0) & (n < N))
        and len(np.unique(a * N + n)) == T
    )
    if not ok:
        raise NotImplementedError("inputs violate mined-triplet structure")

    # palette capacity = max distinct positives used by any anchor row
    ap_pairs = np.unique(a * N + p)
    npal = np.bincount(ap_pairs // N, minlength=N)
    s_max = int(npal.max())
    s_pad = max(2, s_max + (s_max & 1))

    nc = _get_program(s_pad)
    in_maps = _shard_inputs(samples, targets, a, p, n, s_pad)
    res = run_bass_kernel_spmd(nc, in_maps, list(range(NCORES)), trace=_want_trace)
    total = sum(float(res.results[c]["out"][0, 0]) for c in range(NCORES))
    loss = np.float32(total / T)
    if _want_trace:
        return loss, res
    return loss
